# revision 11
# baseline (speedup 1.0000x reference)
"""Trainium2 Bass kernel for ConditionalScoreDecoder diffusion sampling.

Contract: kernel(**inputs) takes the FULL inputs of reference.setup_inputs()
and returns the FULL (4096, 64) float32 output. Internally shards the
B*num_samples = 262144 row dimension across 8 NeuronCores (pure data
parallel), replicating the tiny MLP weights.

Algebraic restructuring of the score net (exactly equivalent up to fp32
rounding / association):
    h0 = silu(x*wx + [z,cti,c] @ Wzc + (bi + s_i*ws))
    u1 = silu(h0 @ A1 + a1)
    u2 = silu(h0 @ A2 + u1 @ (B1@A2) + (a2 + b1@A2))
    score = h0 @ Wo + u1 @ (B1@Wo) + u2 @ (B2@Wo) + c_s
        with c_s = (b1+b2) @ Wo + bo
(residual adds are folded away, so no h1/h2 materialization on device)

Diffusion update per scan step t (i = 99-t):
    x_new = k0*x + k1*(score_raw + c_s) + k2*noise_i
          = k0*x + k1*score_raw + noise_dev[t]
    noise_dev[t] = k1*c_s + k2*noise_i  (host precomputed, k2[i=0] = 0)

On-device layout: activations kept transposed (features on partitions, rows
on the free dim). x state lives as row 0 of the per-tile [71, T] input tile
(row 0 = x, rows 1..70 = z|cti|c features). Per-step noise is staged into
free-dim chunks on partition 0.
"""

import os
import sys

for _p in ("/opt/trn_rl_repo", "/root/.axon_site/_ro/trn_rl_repo"):
    if os.path.isdir(_p) and _p not in sys.path:
        sys.path.insert(0, _p)

import numpy as np

import concourse.bacc as bacc
import concourse.bass as bass
import concourse.tile as tile
from concourse import mybir
from concourse.bass_utils import run_bass_kernel_spmd

# ---- problem constants (hardcoded per spec) ----
B = 4096
S = 64
N = B * S            # 262144
CORES = 8
R = N // CORES       # 32768 rows per core
LATENT = 64
COV = 5
NCOND = LATENT + 1 + COV   # 70
HID = 256
STEPS = 100
BETA_START = 1e-4
BETA_END = 0.02

# ---- kernel tiling config ----
T = 512              # rows per tile (matmul moving free dim)
NT = R // T          # 64 tiles per core
G = 2                # tiles processed per loop-body iteration
NCH = 10             # noise steps per staging chunk

F32 = mybir.dt.float32

# matmul operand dtype: "float32" (exact, 4 cyc/row) or "float32r" (1 cyc/row)
MM_DT_NAME = os.environ.get("KERNEL_MM_DT", "float32r")

_PROG_CACHE = {}


# ------------------------------------------------------------------
# host-side precompute
# ------------------------------------------------------------------

def _np(x):
    return np.asarray(x, dtype=np.float32)


def schedule():
    betas = np.linspace(BETA_START, BETA_END, STEPS).astype(np.float32)
    alphas = (1.0 - betas).astype(np.float32)
    abar = np.cumprod(alphas, dtype=np.float32)
    b = betas.astype(np.float64)
    a = alphas.astype(np.float64)
    ab = abar.astype(np.float64)
    # scan order: t = 0..99 maps to i = 99..0
    idx = np.arange(STEPS - 1, -1, -1)
    k0 = 1.0 / np.sqrt(a[idx])
    k1 = b[idx] * np.sqrt(1.0 - ab[idx]) / np.sqrt(a[idx])
    k2 = np.where(idx > 0, np.sqrt(b[idx]), 0.0)
    return idx, k0, k1, k2


def prep_weights(params):
    """Returns dict of host arrays for the device program."""
    Wi = _np(params["inp"]["W"])        # (72, 256)
    bi = _np(params["inp"]["b"])        # (256,)
    A1 = _np(params["res"][0]["l1"]["W"])
    a1 = _np(params["res"][0]["l1"]["b"])
    B1 = _np(params["res"][0]["l2"]["W"])
    b1 = _np(params["res"][0]["l2"]["b"])
    A2 = _np(params["res"][1]["l1"]["W"])
    a2 = _np(params["res"][1]["l1"]["b"])
    B2 = _np(params["res"][1]["l2"]["W"])
    b2 = _np(params["res"][1]["l2"]["b"])
    Wo = _np(params["out"]["W"])        # (256, 1)
    bo = _np(params["out"]["b"])        # (1,)

    wx = Wi[0]                          # (256,)
    ws = Wi[1]                          # (256,)
    Wzc = Wi[2:]                        # (70, 256)

    C12 = (B1.astype(np.float64) @ A2.astype(np.float64)).astype(np.float32)
    c2 = (a2.astype(np.float64) + b1.astype(np.float64) @ A2.astype(np.float64)).astype(np.float32)
    w1 = (B1.astype(np.float64) @ Wo.astype(np.float64)).astype(np.float32)[:, 0]
    w2 = (B2.astype(np.float64) @ Wo.astype(np.float64)).astype(np.float32)[:, 0]
    c_s = float(bo[0] + (b1 + b2).astype(np.float64) @ Wo.astype(np.float64)[:, 0])

    idx, k0, k1, k2 = schedule()
    s_vals = idx.astype(np.float64) / STEPS
    # bias per scan-step for the input-layer silu: bi + s*ws   -> [steps, 256]
    bias_steps = (bi[None, :].astype(np.float64) + s_vals[:, None] * ws[None, :]).astype(np.float32)

    # Input-tile row layout (DVE writes must start at partition 0 or 32):
    #   row 0 = x_hi, row 32 = x_lo, rows 1..31 = cond[0:31], rows 33..71 = cond[31:70]
    # x enters as a hi/lo pair so the fp32 state survives the reduced-precision
    # matmul path exactly. w_inp rows are permuted to match.
    w_inp = np.empty((72, 256), np.float32)
    w_inp[0] = wx
    w_inp[32] = wx
    w_inp[1:32] = Wzc[0:31]
    w_inp[33:72] = Wzc[31:70]
    wcat = np.stack(
        [Wo[0:128, 0], Wo[128:256, 0], w1[0:128], w1[128:256], w2[0:128], w2[128:256]],
        axis=1,
    ).astype(np.float32)                                       # (128, 6)

    return {
        "w_inp": np.ascontiguousarray(w_inp),
        "a1k0": np.ascontiguousarray(A1[0:128, :]),
        "a1k1": np.ascontiguousarray(A1[128:256, :]),
        "u2k0": np.ascontiguousarray(A2[0:128, :]),
        "u2k1": np.ascontiguousarray(A2[128:256, :]),
        "u2k2": np.ascontiguousarray(C12[0:128, :]),
        "u2k3": np.ascontiguousarray(C12[128:256, :]),
        "wcat": np.ascontiguousarray(wcat),
        "bs0": np.ascontiguousarray(bias_steps[:, 0:128].T),   # (128, steps)
        "bs1": np.ascontiguousarray(bias_steps[:, 128:256].T),
        "a1b": np.ascontiguousarray(np.stack([a1[0:128], a1[128:256]], axis=1)),  # (128,2)
        "c2b": np.ascontiguousarray(np.stack([c2[0:128], c2[128:256]], axis=1)),
        "c_s": c_s,
        "k0": k0,
        "k1": k1,
        "k2": k2,
    }


def prep_noise(c_s, k1, k2, steps=STEPS, n=N):
    """noise_dev[t] = k1[t]*c_s + k2[t]*normal(fold_in(key42, i_t), (n,))
    and x0 = normal(fold_in(key42, 100), (n,)). Bit-exact jax threefry on CPU."""
    import jax

    cpu = jax.devices("cpu")[0]
    idx = np.arange(STEPS - 1, -1, -1)[:steps]
    with jax.default_device(cpu):
        key = jax.random.key(42)
        x0 = np.asarray(jax.random.normal(jax.random.fold_in(key, STEPS), (n,), jax.numpy.float32))
        noise = np.empty((steps, n), dtype=np.float32)
        for t in range(steps):
            i = int(idx[t])
            nz = jax.random.normal(jax.random.fold_in(key, i), (n,), jax.numpy.float32)
            noise[t] = np.asarray(nz) * np.float32(k2[t]) + np.float32(k1[t] * c_s)
    return x0, noise


def prep_cond(z_t, cti_t, c_t, num_samples):
    cond = np.concatenate([_np(z_t).T, _np(cti_t).T, _np(c_t).T], axis=0)  # (70, B)
    return np.repeat(cond, num_samples, axis=1)                            # (70, N)


# ------------------------------------------------------------------
# device program
# ------------------------------------------------------------------

def build_program(n_tiles=NT, steps=STEPS, g=G, t_sz=T, mm_dt_name=MM_DT_NAME,
                  k1=None, k0=None, num_cores=CORES):
    """Builds + compiles the SPMD single-core program."""
    assert n_tiles % g == 0
    mm_dt = getattr(mybir.dt, mm_dt_name)
    rows = n_tiles * t_sz

    MMD = mm_dt      # dtype of matmul-feeding tensors
    def vw(ap):
        return ap

    nc = bacc.Bacc("TRN2", target_bir_lowering=False, debug=False,
                   num_devices=num_cores)

    d = {}
    def din(name, shape, dt=F32):
        d[name] = nc.dram_tensor(name, list(shape), dt, kind="ExternalInput").ap()
    din("cond", (NCOND, rows), MMD)
    din("x0", (1, rows))
    din("noise", (n_tiles, steps * t_sz))
    din("w_inp", (72, 256), MMD)
    for nm in ("a1k0", "a1k1", "u2k0", "u2k1", "u2k2", "u2k3"):
        din(nm, (128, 256), MMD)
    din("wcat", (128, 6), MMD)
    din("bs0", (128, steps))
    din("bs1", (128, steps))
    din("a1b", (128, 2))
    din("c2b", (128, 2))
    out_ap = nc.dram_tensor("out", [1, rows], F32, kind="ExternalOutput").ap()

    Silu = mybir.ActivationFunctionType.Silu
    MULT = mybir.AluOpType.mult
    ADD = mybir.AluOpType.add

    with tile.TileContext(nc) as tc:
        with (
            tc.tile_pool(name="wp", bufs=1) as wp,
            tc.tile_pool(name="state", bufs=1) as state,
            tc.tile_pool(name="nstp", bufs=2) as nstp,
            tc.tile_pool(name="hp", bufs=3) as hp,
            tc.tile_pool(name="xp", bufs=4) as xp,
            tc.tile_pool(name="pp_pre", bufs=2, space="PSUM") as pp_pre,
            tc.tile_pool(name="pp_u1", bufs=2, space="PSUM") as pp_u1,
            tc.tile_pool(name="pp_u2", bufs=2, space="PSUM") as pp_u2,
            tc.tile_pool(name="pp_s", bufs=2, space="PSUM") as pp_s,
        ):
            # --- load weights/constants once ---
            w_sb = {}
            for nm, shape, wdt in (
                ("w_inp", (72, 256), MMD),
                ("a1k0", (128, 256), MMD), ("a1k1", (128, 256), MMD),
                ("u2k0", (128, 256), MMD), ("u2k1", (128, 256), MMD),
                ("u2k2", (128, 256), MMD), ("u2k3", (128, 256), MMD),
                ("wcat", (128, 6), MMD),
                ("bs0", (128, steps), F32), ("bs1", (128, steps), F32),
                ("a1b", (128, 2), F32), ("c2b", (128, 2), F32),
            ):
                w_sb[nm] = wp.tile(list(shape), wdt, tag=nm, name=nm)
                nc.sync.dma_start(w_sb[nm][:], d[nm][:])
            bs = (w_sb["bs0"], w_sb["bs1"])

            with tc.For_i(0, n_tiles // g, 1) as gi:
                inp = []
                xfull = []
                nst = [None] * g
                for t in range(g):
                    col = gi * (g * t_sz) + t * t_sz
                    it = state.tile([72, t_sz], MMD, tag=f"inp{t}", name=f"inp{t}")
                    nc.sync.dma_start(it[1:32, :], d["cond"][0:31, bass.ds(col, t_sz)])
                    nc.sync.dma_start(it[33:72, :], d["cond"][31:70, bass.ds(col, t_sz)])
                    xf = state.tile([1, t_sz], F32, tag=f"xf{t}", name=f"xf{t}")
                    nc.sync.dma_start(xf[0:1, :], d["x0"][:, bass.ds(col, t_sz)])
                    # split exact x into hi/lo rows of the input tile
                    nc.vector.tensor_copy(it[0:1, :], xf[0:1, :])
                    nc.vector.tensor_sub(it[32:33, :], xf[0:1, :], it[0:1, :])
                    inp.append(it)
                    xfull.append(xf)

                nch = min(NCH, steps)
                for step in range(steps):
                    for t in range(g):
                        # stage noise chunk (free-dim layout on partition 0)
                        if step % nch == 0:
                            clen = min(nch, steps - step)
                            nst[t] = nstp.tile([1, nch * t_sz], F32, tag=f"nst{t}", name=f"nst{t}")
                            nc.sync.dma_start(
                                nst[t][0:1, 0:clen * t_sz],
                                d["noise"][bass.ds(gi * g + t, 1),
                                           step * t_sz:(step + clen) * t_sz],
                            )
                        noise_row = nst[t][0:1, (step % nch) * t_sz:(step % nch + 1) * t_sz]

                        # input layer: pre0[m] = W_aug[:,m].T @ inp  (K=71)
                        h0 = []
                        for mi in range(2):
                            ms = slice(mi * 128, (mi + 1) * 128)
                            pre = pp_pre.tile([128, t_sz], F32, tag="pre", name="pre")
                            nc.tensor.matmul(pre[:], vw(w_sb["w_inp"][:, ms]),
                                             vw(inp[t][0:72, :]), start=True, stop=True)
                            h = hp.tile([128, t_sz], MMD, tag=f"h0_{mi}", name=f"h0_{mi}")
                            nc.scalar.activation(h[:], pre[:], Silu,
                                                 bias=bs[mi][:, step:step + 1])
                            h0.append(h)

                        # u1 = silu(h0 @ A1 + a1)
                        u1 = []
                        for mi in range(2):
                            ms = slice(mi * 128, (mi + 1) * 128)
                            pu = pp_u1.tile([128, t_sz], F32, tag="u1", name="u1p")
                            nc.tensor.matmul(pu[:], vw(w_sb["a1k0"][:, ms]),
                                             vw(h0[0][:]), start=True, stop=False)
                            nc.tensor.matmul(pu[:], vw(w_sb["a1k1"][:, ms]),
                                             vw(h0[1][:]), start=False, stop=True)
                            u = hp.tile([128, t_sz], MMD, tag=f"u1_{mi}", name=f"u1_{mi}")
                            nc.scalar.activation(u[:], pu[:], Silu,
                                                 bias=w_sb["a1b"][:, mi:mi + 1])
                            u1.append(u)

                        # u2 = silu(h0 @ A2 + u1 @ C12 + c2)
                        u2 = []
                        srcs = (h0[0], h0[1], u1[0], u1[1])
                        wk = ("u2k0", "u2k1", "u2k2", "u2k3")
                        for mi in range(2):
                            ms = slice(mi * 128, (mi + 1) * 128)
                            pu = pp_u2.tile([128, t_sz], F32, tag="u2", name="u2p")
                            for ki in range(4):
                                nc.tensor.matmul(pu[:], vw(w_sb[wk[ki]][:, ms]),
                                                 vw(srcs[ki][:]),
                                                 start=(ki == 0), stop=(ki == 3))
                            u = hp.tile([128, t_sz], MMD, tag=f"u2_{mi}", name=f"u2_{mi}")
                            nc.scalar.activation(u[:], pu[:], Silu,
                                                 bias=w_sb["c2b"][:, mi:mi + 1])
                            u2.append(u)

                        # score_raw = [h0,u1,u2] @ wcat   (M=1)
                        ps = pp_s.tile([1, t_sz], F32, tag="s", name="sp")
                        ssrc = (h0[0], h0[1], u1[0], u1[1], u2[0], u2[1])
                        for j in range(6):
                            nc.tensor.matmul(ps[0:1, :], vw(w_sb["wcat"][:, j:j + 1]),
                                             vw(ssrc[j][:]),
                                             start=(j == 0), stop=(j == 5))

                        # x_new = k1*score + (k0*x + noise_dev)   (exact fp32 on DVE)
                        t1 = xp.tile([1, t_sz], F32, tag="t1", name="t1")
                        nc.vector.scalar_tensor_tensor(t1[:], xfull[t][0:1, :],
                                                       float(k0[step]), noise_row,
                                                       MULT, ADD)
                        nc.vector.scalar_tensor_tensor(xfull[t][0:1, :], ps[0:1, :],
                                                       float(k1[step]), t1[:],
                                                       MULT, ADD)
                        # refresh hi/lo rows for the next step's input matmul
                        nc.vector.tensor_copy(inp[t][0:1, :], xfull[t][0:1, :])
                        nc.vector.tensor_sub(inp[t][32:33, :], xfull[t][0:1, :],
                                             inp[t][0:1, :])

                for t in range(g):
                    col = gi * (g * t_sz) + t * t_sz
                    nc.sync.dma_start(out_ap[:, bass.ds(col, t_sz)],
                                      xfull[t][0:1, :])

    nc.compile()
    return nc


# ------------------------------------------------------------------
# public entry point
# ------------------------------------------------------------------

def _get_program():
    key = (NT, STEPS, G, T, MM_DT_NAME)
    if key not in _PROG_CACHE:
        _, k0, k1, _ = schedule()
        _PROG_CACHE[key] = build_program(k1=k1, k0=k0)
    return _PROG_CACHE[key]


def make_in_maps(z_t, cti_t, c_t, num_samples, params, n_tiles=NT, steps=STEPS):
    w = prep_weights(params)
    x0, noise = prep_noise(w["c_s"], w["k1"], w["k2"], steps=steps)
    cond = prep_cond(z_t, cti_t, c_t, num_samples)

    in_maps = []
    rows = n_tiles * T
    for c in range(CORES):
        sl = slice(c * R, c * R + rows)
        nz = noise[:, sl].reshape(steps, n_tiles, T).transpose(1, 0, 2).reshape(n_tiles, steps * T)
        m = {
            "cond": np.ascontiguousarray(cond[:, sl]),
            "x0": np.ascontiguousarray(x0[None, sl]),
            "noise": np.ascontiguousarray(nz),
        }
        for nm in ("w_inp", "a1k0", "a1k1", "u2k0", "u2k1", "u2k2", "u2k3",
                   "wcat", "bs0", "bs1", "a1b", "c2b"):
            m[nm] = w[nm][:, :steps] if nm in ("bs0", "bs1") else w[nm]
        in_maps.append(m)
    return in_maps


def kernel(z_t, cti_t, c_t, num_samples, params, _run_kwargs=None):
    num_samples = int(num_samples)
    assert _np(z_t).shape == (B, LATENT) and num_samples == S, "shapes are hardcoded"

    nc = _get_program()
    in_maps = make_in_maps(z_t, cti_t, c_t, num_samples, params)
    res = run_bass_kernel_spmd(nc, in_maps, list(range(CORES)), **(_run_kwargs or {}))
    out = np.concatenate([res.results[c]["out"][0] for c in range(CORES)])
    kernel.last_result = res
    return out.reshape(B, S).astype(np.float32)


# revision 17
# speedup vs baseline: 1.0332x; 1.0332x over previous
"""Trainium2 Bass kernel for ConditionalScoreDecoder diffusion sampling.

Contract: kernel(**inputs) takes the FULL inputs of reference.setup_inputs()
and returns the FULL (4096, 64) float32 output. Internally shards the
B*num_samples = 262144 row dimension across 8 NeuronCores (pure data
parallel), replicating the tiny MLP weights.

Algebraic restructuring of the score net (exactly equivalent up to fp32
rounding / association):
    h0 = silu(x*wx + [z,cti,c] @ Wzc + (bi + s_i*ws))
    u1 = silu(h0 @ A1 + a1)
    u2 = silu(h0 @ A2 + u1 @ (B1@A2) + (a2 + b1@A2))
    score = h0 @ Wo + u1 @ (B1@Wo) + u2 @ (B2@Wo) + c_s
        with c_s = (b1+b2) @ Wo + bo
(residual adds are folded away, so no h1/h2 materialization on device)

Diffusion update per scan step t (i = 99-t):
    x_new = k0*x + k1*(score_raw + c_s) + k2*noise_i
          = k0*x + k1*score_raw + noise_dev[t]
    noise_dev[t] = k1*c_s + k2*noise_i  (host precomputed, k2[i=0] = 0)

On-device layout: activations kept transposed (features on partitions, rows
on the free dim). x state lives as row 0 of the per-tile [71, T] input tile
(row 0 = x, rows 1..70 = z|cti|c features). Per-step noise is staged into
free-dim chunks on partition 0.
"""

import os
import sys

for _p in ("/opt/trn_rl_repo", "/root/.axon_site/_ro/trn_rl_repo"):
    if os.path.isdir(_p) and _p not in sys.path:
        sys.path.insert(0, _p)

import numpy as np

import concourse.bacc as bacc
import concourse.bass as bass
import concourse.tile as tile
from concourse import mybir
from concourse.bass_utils import run_bass_kernel_spmd

# ---- problem constants (hardcoded per spec) ----
B = 4096
S = 64
N = B * S            # 262144
CORES = 8
R = N // CORES       # 32768 rows per core
LATENT = 64
COV = 5
NCOND = LATENT + 1 + COV   # 70
HID = 256
STEPS = 100
BETA_START = 1e-4
BETA_END = 0.02

# ---- kernel tiling config ----
T = 512              # rows per tile (matmul moving free dim)
NT = R // T          # 64 tiles per core
G = 2                # tiles processed per loop-body iteration
NCH = 10             # noise steps per staging chunk

F32 = mybir.dt.float32

# matmul operand dtype: "float32" (exact, 4 cyc/row) or "float32r" (1 cyc/row)
MM_DT_NAME = os.environ.get("KERNEL_MM_DT", "float32r")
# dtype of the res-block/score path (weights + activations): bfloat16 or same as MM_DT
ACT_DT_NAME = os.environ.get("KERNEL_ACT_DT", "float32r")
COLT = int(os.environ.get("KERNEL_COLT", "0"))       # col-tile the score matmuls
STAG = int(os.environ.get("KERNEL_STAG", "0"))       # staggered loop reset

_PROG_CACHE = {}


# ------------------------------------------------------------------
# host-side precompute
# ------------------------------------------------------------------

def _np(x):
    return np.asarray(x, dtype=np.float32)


def schedule():
    betas = np.linspace(BETA_START, BETA_END, STEPS).astype(np.float32)
    alphas = (1.0 - betas).astype(np.float32)
    abar = np.cumprod(alphas, dtype=np.float32)
    b = betas.astype(np.float64)
    a = alphas.astype(np.float64)
    ab = abar.astype(np.float64)
    # scan order: t = 0..99 maps to i = 99..0
    idx = np.arange(STEPS - 1, -1, -1)
    k0 = 1.0 / np.sqrt(a[idx])
    k1 = b[idx] * np.sqrt(1.0 - ab[idx]) / np.sqrt(a[idx])
    k2 = np.where(idx > 0, np.sqrt(b[idx]), 0.0)
    return idx, k0, k1, k2


def prep_weights(params):
    """Returns dict of host arrays for the device program."""
    Wi = _np(params["inp"]["W"])        # (72, 256)
    bi = _np(params["inp"]["b"])        # (256,)
    A1 = _np(params["res"][0]["l1"]["W"])
    a1 = _np(params["res"][0]["l1"]["b"])
    B1 = _np(params["res"][0]["l2"]["W"])
    b1 = _np(params["res"][0]["l2"]["b"])
    A2 = _np(params["res"][1]["l1"]["W"])
    a2 = _np(params["res"][1]["l1"]["b"])
    B2 = _np(params["res"][1]["l2"]["W"])
    b2 = _np(params["res"][1]["l2"]["b"])
    Wo = _np(params["out"]["W"])        # (256, 1)
    bo = _np(params["out"]["b"])        # (1,)

    wx = Wi[0]                          # (256,)
    ws = Wi[1]                          # (256,)
    Wzc = Wi[2:]                        # (70, 256)

    C12 = (B1.astype(np.float64) @ A2.astype(np.float64)).astype(np.float32)
    c2 = (a2.astype(np.float64) + b1.astype(np.float64) @ A2.astype(np.float64)).astype(np.float32)
    w1 = (B1.astype(np.float64) @ Wo.astype(np.float64)).astype(np.float32)[:, 0]
    w2 = (B2.astype(np.float64) @ Wo.astype(np.float64)).astype(np.float32)[:, 0]
    c_s = float(bo[0] + (b1 + b2).astype(np.float64) @ Wo.astype(np.float64)[:, 0])

    idx, k0, k1, k2 = schedule()
    s_vals = idx.astype(np.float64) / STEPS
    # bias per scan-step for the input-layer silu: bi + s*ws   -> [steps, 256]
    bias_steps = (bi[None, :].astype(np.float64) + s_vals[:, None] * ws[None, :]).astype(np.float32)

    # Input-tile row layout (DVE writes must start at partition 0 or 32):
    #   row 0 = x_hi, row 32 = x_lo, rows 1..31 = cond[0:31], rows 33..71 = cond[31:70]
    # x enters as a hi/lo pair so the fp32 state survives the reduced-precision
    # matmul path exactly. w_inp rows are permuted to match.
    w_inp = np.empty((72, 256), np.float32)
    w_inp[0] = wx
    w_inp[32] = wx
    w_inp[1:32] = Wzc[0:31]
    w_inp[33:72] = Wzc[31:70]
    # score weight chunks padded to M=32 (col-tiling needs 32-wide dst groups);
    # column j*32 holds the real weight vector, the rest are zeros
    wcols = [Wo[0:128, 0], Wo[128:256, 0], w1[0:128], w1[128:256], w2[0:128], w2[128:256]]
    wcat = np.zeros((128, 6 * 32), np.float32)
    for j, wc in enumerate(wcols):
        wcat[:, j * 32] = wc

    return {
        "w_inp": np.ascontiguousarray(w_inp),
        "a1k0": np.ascontiguousarray(A1[0:128, :]),
        "a1k1": np.ascontiguousarray(A1[128:256, :]),
        "u2k0": np.ascontiguousarray(A2[0:128, :]),
        "u2k1": np.ascontiguousarray(A2[128:256, :]),
        "u2k2": np.ascontiguousarray(C12[0:128, :]),
        "u2k3": np.ascontiguousarray(C12[128:256, :]),
        "wcat": np.ascontiguousarray(wcat),
        "bs0": np.ascontiguousarray(bias_steps[:, 0:128].T),   # (128, steps)
        "bs1": np.ascontiguousarray(bias_steps[:, 128:256].T),
        "a1b": np.ascontiguousarray(np.stack([a1[0:128], a1[128:256]], axis=1)),  # (128,2)
        "c2b": np.ascontiguousarray(np.stack([c2[0:128], c2[128:256]], axis=1)),
        "c_s": c_s,
        "k0": k0,
        "k1": k1,
        "k2": k2,
    }


def prep_noise(c_s, k1, k2, steps=STEPS, n=N):
    """noise_dev[t] = k1[t]*c_s + k2[t]*normal(fold_in(key42, i_t), (n,))
    and x0 = normal(fold_in(key42, 100), (n,)). Bit-exact jax threefry on CPU."""
    import jax

    cpu = jax.devices("cpu")[0]
    idx = np.arange(STEPS - 1, -1, -1)[:steps]
    with jax.default_device(cpu):
        key = jax.random.key(42)
        x0 = np.asarray(jax.random.normal(jax.random.fold_in(key, STEPS), (n,), jax.numpy.float32))
        noise = np.empty((steps, n), dtype=np.float32)
        for t in range(steps):
            i = int(idx[t])
            nz = jax.random.normal(jax.random.fold_in(key, i), (n,), jax.numpy.float32)
            noise[t] = np.asarray(nz) * np.float32(k2[t]) + np.float32(k1[t] * c_s)
    return x0, noise


def prep_cond(z_t, cti_t, c_t, num_samples):
    cond = np.concatenate([_np(z_t).T, _np(cti_t).T, _np(c_t).T], axis=0)  # (70, B)
    return np.repeat(cond, num_samples, axis=1)                            # (70, N)


# ------------------------------------------------------------------
# device program
# ------------------------------------------------------------------

def build_program(n_tiles=NT, steps=STEPS, g=G, t_sz=T, mm_dt_name=MM_DT_NAME,
                  act_dt_name=ACT_DT_NAME, colt=COLT, stag=STAG,
                  k1=None, k0=None, num_cores=CORES):
    """Builds + compiles the SPMD single-core program."""
    assert n_tiles % g == 0
    mm_dt = getattr(mybir.dt, mm_dt_name)
    rows = n_tiles * t_sz

    MMD = mm_dt                               # input-layer path (x hi/lo + cond)
    ACTD = getattr(mybir.dt, act_dt_name)     # res/score path
    def vw(ap):
        return ap

    nc = bacc.Bacc("TRN2", target_bir_lowering=False, debug=False,
                   num_devices=num_cores)

    d = {}
    def din(name, shape, dt=F32):
        d[name] = nc.dram_tensor(name, list(shape), dt, kind="ExternalInput").ap()
    din("cond", (NCOND, rows), MMD)
    din("x0", (1, rows))
    din("noise", (n_tiles, steps * t_sz))
    din("w_inp", (72, 256), MMD)
    for nm in ("a1k0", "a1k1", "u2k0", "u2k1", "u2k2", "u2k3"):
        din(nm, (128, 256), ACTD)
    din("wcat", (128, 6 * 32), ACTD)
    din("bs0", (128, steps))
    din("bs1", (128, steps))
    din("a1b", (128, 2))
    din("c2b", (128, 2))
    out_ap = nc.dram_tensor("out", [1, rows], F32, kind="ExternalOutput").ap()

    Silu = mybir.ActivationFunctionType.Silu
    MULT = mybir.AluOpType.mult
    ADD = mybir.AluOpType.add

    with tile.TileContext(nc) as tc:
        with (
            tc.tile_pool(name="wp", bufs=1) as wp,
            tc.tile_pool(name="state", bufs=1) as state,
            tc.tile_pool(name="nstp", bufs=2) as nstp,
            tc.tile_pool(name="hp", bufs=3) as hp,
            tc.tile_pool(name="xp", bufs=4) as xp,
            tc.tile_pool(name="pp_pre", bufs=2, space="PSUM") as pp_pre,
            tc.tile_pool(name="pp_u1", bufs=2, space="PSUM") as pp_u1,
            tc.tile_pool(name="pp_u2", bufs=2, space="PSUM") as pp_u2,
            tc.tile_pool(name="pp_s", bufs=2, space="PSUM") as pp_s,
        ):
            # --- load weights/constants once ---
            w_sb = {}
            for nm, shape, wdt in (
                ("w_inp", (72, 256), MMD),
                ("a1k0", (128, 256), ACTD), ("a1k1", (128, 256), ACTD),
                ("u2k0", (128, 256), ACTD), ("u2k1", (128, 256), ACTD),
                ("u2k2", (128, 256), ACTD), ("u2k3", (128, 256), ACTD),
                ("wcat", (128, 6 * 32), ACTD),
                ("bs0", (128, steps), F32), ("bs1", (128, steps), F32),
                ("a1b", (128, 2), F32), ("c2b", (128, 2), F32),
            ):
                w_sb[nm] = wp.tile(list(shape), wdt, tag=nm, name=nm)
                nc.sync.dma_start(w_sb[nm][:], d[nm][:])
            bs = (w_sb["bs0"], w_sb["bs1"])

            with tc.For_i(0, n_tiles // g, 1, staggered_reset=bool(stag)) as gi:
                inp = []
                xfull = []
                nst = [None] * g
                for t in range(g):
                    col = gi * (g * t_sz) + t * t_sz
                    it = state.tile([72, t_sz], MMD, tag=f"inp{t}", name=f"inp{t}")
                    nc.sync.dma_start(it[1:32, :], d["cond"][0:31, bass.ds(col, t_sz)])
                    nc.sync.dma_start(it[33:72, :], d["cond"][31:70, bass.ds(col, t_sz)])
                    xf = state.tile([1, t_sz], F32, tag=f"xf{t}", name=f"xf{t}")
                    nc.sync.dma_start(xf[0:1, :], d["x0"][:, bass.ds(col, t_sz)])
                    # split exact x into hi/lo rows of the input tile
                    nc.vector.tensor_copy(it[0:1, :], xf[0:1, :])
                    nc.vector.tensor_sub(it[32:33, :], xf[0:1, :], it[0:1, :])
                    inp.append(it)
                    xfull.append(xf)

                nch = min(NCH, steps)
                for step in range(steps):
                    for t in range(g):
                        # stage noise chunk (free-dim layout on partition 0)
                        if step % nch == 0:
                            clen = min(nch, steps - step)
                            nst[t] = nstp.tile([1, nch * t_sz], F32, tag=f"nst{t}", name=f"nst{t}")
                            nc.sync.dma_start(
                                nst[t][0:1, 0:clen * t_sz],
                                d["noise"][bass.ds(gi * g + t, 1),
                                           step * t_sz:(step + clen) * t_sz],
                            )
                        noise_row = nst[t][0:1, (step % nch) * t_sz:(step % nch + 1) * t_sz]

                        # input layer: pre0[m] = W_aug[:,m].T @ inp  (K=71)
                        h0 = []
                        for mi in range(2):
                            ms = slice(mi * 128, (mi + 1) * 128)
                            pre = pp_pre.tile([128, t_sz], F32, tag="pre", name="pre")
                            nc.tensor.matmul(pre[:], vw(w_sb["w_inp"][:, ms]),
                                             vw(inp[t][0:72, :]), start=True, stop=True)
                            h = hp.tile([128, t_sz], ACTD, tag=f"h0_{mi}", name=f"h0_{mi}")
                            nc.scalar.activation(h[:], pre[:], Silu,
                                                 bias=bs[mi][:, step:step + 1])
                            h0.append(h)

                        # u1 = silu(h0 @ A1 + a1)
                        u1 = []
                        for mi in range(2):
                            ms = slice(mi * 128, (mi + 1) * 128)
                            pu = pp_u1.tile([128, t_sz], F32, tag="u1", name="u1p")
                            nc.tensor.matmul(pu[:], vw(w_sb["a1k0"][:, ms]),
                                             vw(h0[0][:]), start=True, stop=False)
                            nc.tensor.matmul(pu[:], vw(w_sb["a1k1"][:, ms]),
                                             vw(h0[1][:]), start=False, stop=True)
                            u = hp.tile([128, t_sz], ACTD, tag=f"u1_{mi}", name=f"u1_{mi}")
                            nc.scalar.activation(u[:], pu[:], Silu,
                                                 bias=w_sb["a1b"][:, mi:mi + 1])
                            u1.append(u)

                        # u2 = silu(h0 @ A2 + u1 @ C12 + c2)
                        u2 = []
                        srcs = (h0[0], h0[1], u1[0], u1[1])
                        wk = ("u2k0", "u2k1", "u2k2", "u2k3")
                        for mi in range(2):
                            ms = slice(mi * 128, (mi + 1) * 128)
                            pu = pp_u2.tile([128, t_sz], F32, tag="u2", name="u2p")
                            for ki in range(4):
                                nc.tensor.matmul(pu[:], vw(w_sb[wk[ki]][:, ms]),
                                                 vw(srcs[ki][:]),
                                                 start=(ki == 0), stop=(ki == 3))
                            u = hp.tile([128, t_sz], ACTD, tag=f"u2_{mi}", name=f"u2_{mi}")
                            nc.scalar.activation(u[:], pu[:], Silu,
                                                 bias=w_sb["c2b"][:, mi:mi + 1])
                            u2.append(u)

                        # score_raw = [h0,u1,u2] @ wcat   (M=1)
                        ssrc = (h0[0], h0[1], u1[0], u1[1], u2[0], u2[1])
                        if colt:
                            # 3 rounds of 2 col-tiled concurrent matmuls (M=32,
                            # only output row 32j is real); partials land on
                            # psum partitions 0 and 32
                            ps = pp_s.tile([64, t_sz], F32, tag="s", name="sp")
                            for r in range(3):
                                nc.tensor.matmul(ps[0:32, :],
                                                 vw(w_sb["wcat"][:, 64*r:64*r+32]),
                                                 vw(ssrc[2*r][:]),
                                                 start=(r == 0), stop=(r == 2),
                                                 tile_position=(0, 0))
                                nc.tensor.matmul(ps[32:64, :],
                                                 vw(w_sb["wcat"][:, 64*r+32:64*r+64]),
                                                 vw(ssrc[2*r+1][:]),
                                                 start=(r == 0), stop=(r == 2),
                                                 tile_position=(0, 32))
                        else:
                            ps = pp_s.tile([1, t_sz], F32, tag="s", name="sp")
                            for j in range(6):
                                nc.tensor.matmul(ps[0:1, :],
                                                 vw(w_sb["wcat"][:, 32*j:32*j + 1]),
                                                 vw(ssrc[j][:]),
                                                 start=(j == 0), stop=(j == 5))

                        # x_new = k1*score + (k0*x + noise_dev)   (exact fp32 on DVE)
                        t1 = xp.tile([1, t_sz], F32, tag="t1", name="t1")
                        nc.vector.scalar_tensor_tensor(t1[:], xfull[t][0:1, :],
                                                       float(k0[step]), noise_row,
                                                       MULT, ADD)
                        if colt:
                            t2 = xp.tile([1, t_sz], F32, tag="t2", name="t2")
                            nc.vector.scalar_tensor_tensor(t2[:], ps[32:33, :],
                                                           float(k1[step]), t1[:],
                                                           MULT, ADD)
                            t1 = t2
                        nc.vector.scalar_tensor_tensor(xfull[t][0:1, :], ps[0:1, :],
                                                       float(k1[step]), t1[:],
                                                       MULT, ADD)
                        # refresh hi/lo rows for the next step's input matmul
                        nc.vector.tensor_copy(inp[t][0:1, :], xfull[t][0:1, :])
                        nc.vector.tensor_sub(inp[t][32:33, :], xfull[t][0:1, :],
                                             inp[t][0:1, :])

                for t in range(g):
                    col = gi * (g * t_sz) + t * t_sz
                    nc.sync.dma_start(out_ap[:, bass.ds(col, t_sz)],
                                      xfull[t][0:1, :])

    nc.compile()
    return nc


# ------------------------------------------------------------------
# public entry point
# ------------------------------------------------------------------

def _get_program():
    key = (NT, STEPS, G, T, MM_DT_NAME, ACT_DT_NAME, COLT, STAG)
    if key not in _PROG_CACHE:
        _, k0, k1, _ = schedule()
        _PROG_CACHE[key] = build_program(k1=k1, k0=k0)
    return _PROG_CACHE[key]


def make_in_maps(z_t, cti_t, c_t, num_samples, params, n_tiles=NT, steps=STEPS):
    w = prep_weights(params)
    x0, noise = prep_noise(w["c_s"], w["k1"], w["k2"], steps=steps)
    cond = prep_cond(z_t, cti_t, c_t, num_samples)

    in_maps = []
    rows = n_tiles * T
    for c in range(CORES):
        sl = slice(c * R, c * R + rows)
        nz = noise[:, sl].reshape(steps, n_tiles, T).transpose(1, 0, 2).reshape(n_tiles, steps * T)
        m = {
            "cond": np.ascontiguousarray(cond[:, sl]),
            "x0": np.ascontiguousarray(x0[None, sl]),
            "noise": np.ascontiguousarray(nz),
        }
        actnp = mybir.dt.np(getattr(mybir.dt, ACT_DT_NAME))
        for nm in ("w_inp", "a1k0", "a1k1", "u2k0", "u2k1", "u2k2", "u2k3",
                   "wcat", "bs0", "bs1", "a1b", "c2b"):
            if nm in ("bs0", "bs1"):
                m[nm] = w[nm][:, :steps]
            elif nm in ("a1k0", "a1k1", "u2k0", "u2k1", "u2k2", "u2k3", "wcat"):
                m[nm] = w[nm].astype(actnp)
            else:
                m[nm] = w[nm]
        in_maps.append(m)
    return in_maps


def kernel(z_t, cti_t, c_t, num_samples, params, _run_kwargs=None):
    num_samples = int(num_samples)
    assert _np(z_t).shape == (B, LATENT) and num_samples == S, "shapes are hardcoded"

    nc = _get_program()
    in_maps = make_in_maps(z_t, cti_t, c_t, num_samples, params)
    res = run_bass_kernel_spmd(nc, in_maps, list(range(CORES)), **(_run_kwargs or {}))
    out = np.concatenate([res.results[c]["out"][0] for c in range(CORES)])
    kernel.last_result = res
    return out.reshape(B, S).astype(np.float32)
